# revision 31
# baseline (speedup 1.0000x reference)
"""MinGRU fused kernel for Trainium2, 8-core SPMD.

Math (exact rewrite of the log-space Heinsen-scan reference in linear space):
    hg = x @ W ; hidden, gate = split(hg)
    c = sigmoid(-gate)            # = exp(log_coeffs)
    z = sigmoid(gate)
    g = max(hidden + 0.5, sigmoid(hidden))   # == where(h>=0, h+0.5, sigmoid(h))
    v = z * g
    h_t = c_t * h_{t-1} + v_t     # tensor_tensor_scan on VectorE, fp32 state
    out = h                       # (reference's exp(log_h) == h exactly)

h is a convex-combination recurrence of bounded positives -> numerically safe
in linear space (validated: 6e-4 max rel err vs reference in pure fp32).

Sharding: (batch b, d-half) per core -> 8 cores, no cross-core communication.
Per core: GEMM [4096, 1024] @ [1024, 1024] in fp16 (1 cyc/row on PE, 8x the
mantissa of bf16), out_T orientation (d on partitions, seq on free) so the
matmul output lands directly in scan layout. Output shard is stored d-major
[512, 4096]; the host transposes during the gather.
"""

import numpy as np

import concourse.bass as bass
import concourse.tile as tile
import concourse.mybir as mybir
from concourse import bacc, bass_utils
from concourse.bass import ds, ts

F32 = mybir.dt.float32
F16 = mybir.dt.float16
AF = mybir.ActivationFunctionType
OP = mybir.AluOpType

B, S, D = 4, 4096, 1024
DSH = D // 2          # d-shard per core (two cores per batch)
N_CORES = 8
P = 128               # partitions
KT = D // P           # 8 contraction chunks
MB = DSH // P         # 4 output row-blocks per core
QT = 1024             # seq chunk per iteration
NQ = S // QT          # 4
NB = 512              # matmul moving free dim (one PSUM bank)
EW = QT               # epoch width (best measured: 1024 = 2 PSUM banks)


def _build(
    bench_reps: int | None = None,
    gemm_only: bool = False,
    loads_once: bool = False,
    ew: int = EW,
    interleave_hg: bool = False,
    ep_bufs: int = 2,
    hb_act: bool = False,
):
    nc = bacc.Bacc("TRN2", debug=False, num_devices=N_CORES)
    xT_d = nc.dram_tensor("xT", [D, S], F16, kind="ExternalInput")
    wh_d = nc.dram_tensor("wh", [D, DSH], F16, kind="ExternalInput")
    wg_d = nc.dram_tensor("wg", [D, DSH], F16, kind="ExternalInput")
    out_d = nc.dram_tensor("out", [DSH, S], F32, kind="ExternalOutput")

    xT_t = xT_d.rearrange("(k p) s -> k p s", p=P)    # [8, 128, 4096]
    wh_t = wh_d.rearrange("(k p) n -> k p n", p=P)    # [8, 128, 512]
    wg_t = wg_d.rearrange("(k p) n -> k p n", p=P)

    with tile.TileContext(nc) as tc:
        with (
            tc.tile_pool(name="wpool", bufs=1) as wpool,
            tc.tile_pool(name="xpool", bufs=1) as xpool,
            tc.tile_pool(name="psum", bufs=2, space="PSUM") as psum,
            tc.tile_pool(name="ep", bufs=ep_bufs) as ep,
        ):
            whs = [
                wpool.tile([P, DSH], F16, tag=f"wh{k}", name=f"wh{k}")
                for k in range(KT)
            ]
            wgs = [
                wpool.tile([P, DSH], F16, tag=f"wg{k}", name=f"wg{k}")
                for k in range(KT)
            ]
            for k in range(KT):
                nc.sync.dma_start(out=whs[k][:], in_=wh_t[k])

            xs = [
                xpool.tile([P, S], F16, tag=f"x{k}", name=f"x{k}")
                for k in range(KT)
            ]

            carries = [
                wpool.tile([P, 1], F32, tag=f"carry{m}", name=f"carry{m}")
                for m in range(MB)
            ]

            # scratch operands for PE warm-up matmuls (HAM clock-gate ramps
            # ~4us into sustained activity; keep PE busy during the x-load
            # prologue so the real matmuls run at full clock)
            wu_w = wpool.tile([P, P], F16, tag="wu_w", name="wu_w")
            wu_x = wpool.tile([P, NB], F16, tag="wu_x", name="wu_x")
            nc.vector.memset(wu_w[:], 0.0)
            nc.vector.memset(wu_x[:], 0.0)

            def loads():
                # x loads in first-consumption order. The first epoch pair
                # (m=0) streams x[k][0:EW] per-k; feed it as fine [P, NB]
                # pieces, k-interleaved, columns ascending. Gate weights after
                # the first two columns; second s-half as [P, EW] chunks.
                for j in range(2):
                    for k in range(KT):
                        nc.sync.dma_start(
                            out=xs[k][:, ds(j * NB, NB)],
                            in_=xT_t[k][:, ds(j * NB, NB)],
                        )
                for k in range(KT):
                    nc.sync.dma_start(out=wgs[k][:], in_=wg_t[k])
                for j in range(2, ew // NB):
                    for k in range(KT):
                        nc.sync.dma_start(
                            out=xs[k][:, ds(j * NB, NB)],
                            in_=xT_t[k][:, ds(j * NB, NB)],
                        )
                for q in range(1, S // ew):
                    for k in range(KT):
                        nc.sync.dma_start(
                            out=xs[k][:, ds(q * ew, ew)],
                            in_=xT_t[k][:, ds(q * ew, ew)],
                        )

            if loads_once:
                loads()

            def body(_iv=None):
                if not loads_once:
                    loads()

                wu = psum.tile([P, ew], F32,
                               tag="iph" if interleave_hg else "acc",
                               name="wu", padded_shape=[P, ew])
                for _ in range(24):
                    nc.tensor.matmul(
                        wu[:, ds(0, NB)], lhsT=wu_w[:], rhs=wu_x[:],
                        start=True, stop=True,
                    )


                def gemm_epoch(wt, m, s0, w):
                    """Accumulate wt.T @ x for seq [s0, s0+w) of row-block m.
                    k outer: each LDW feeds w/NB consecutive matmuls."""
                    pt = psum.tile([P, ew], F32, tag="acc", name="acc",
                                   padded_shape=[P, ew])
                    for k in range(KT):
                        for lo in range(0, w, NB):
                            nb = min(NB, w - lo)
                            nc.tensor.matmul(
                                pt[:, ds(lo, nb)],
                                lhsT=wt[k][:, ts(m, P)],
                                rhs=xs[k][:, ds(s0 + lo, nb)],
                                start=(k == 0),
                                stop=(k == KT - 1),
                            )
                    return pt

                def gemm_pair_interleaved(m, s0, w):
                    """Single k-loop over both tensors: 4-bank rotation."""
                    ph = psum.tile([P, ew], F32, tag="iph", name="iph",
                                   padded_shape=[P, ew])
                    pg = psum.tile([P, ew], F32, tag="ipg", name="ipg",
                                   padded_shape=[P, ew])
                    for k in range(KT):
                        for pt, wt in ((ph, whs), (pg, wgs)):
                            for lo in range(0, w, NB):
                                nb = min(NB, w - lo)
                                nc.tensor.matmul(
                                    pt[:, ds(lo, nb)],
                                    lhsT=wt[k][:, ts(m, P)],
                                    rhs=xs[k][:, ds(s0 + lo, nb)],
                                    start=(k == 0),
                                    stop=(k == KT - 1),
                                )
                    return ph, pg

                def epoch_pair(m, s0, w, first, last_of_m):
                    """Hidden + gate epochs and the fused epilogue for seq
                    [s0, s0+w) of row-block m."""
                    r = ds(0, w)
                    if interleave_hg:
                        ph, pg = gemm_pair_interleaved(m, s0, w)
                        if gemm_only:
                            return
                        sh = ep.tile([P, ew], F32, tag="sh", name="sh")
                        hb = ep.tile([P, ew], F32, tag="hb", name="hb")
                        nc.scalar.activation(sh[:, r], ph[:, r], AF.Sigmoid)
                        nc.vector.tensor_scalar_add(hb[:, r], ph[:, r], 0.5)
                    else:
                        ph = gemm_epoch(whs, m, s0, w)
                        if not gemm_only:
                            # drain hidden psum (frees its slot during gate)
                            sh = ep.tile([P, ew], F32, tag="sh", name="sh")
                            hb = ep.tile([P, ew], F32, tag="hb", name="hb")
                            nc.scalar.activation(sh[:, r], ph[:, r], AF.Sigmoid)
                            if hb_act:
                                nc.scalar.activation(
                                    hb[:, r], ph[:, r],
                                    AF.Identity, bias=0.5,
                                )
                            else:
                                nc.vector.tensor_scalar_add(
                                    hb[:, r], ph[:, r], 0.5
                                )
                        pg = gemm_epoch(wgs, m, s0, w)
                    if gemm_only:
                        return
                    z = ep.tile([P, ew], F32, tag="z", name="z")
                    c = ep.tile([P, ew], F32, tag="c", name="c")
                    h = ep.tile([P, ew], F32, tag="h", name="h")
                    nc.scalar.activation(z[:, r], pg[:, r], AF.Sigmoid)
                    # c = 1 - z == sigmoid(-gate), on DVE (2x mode)
                    nc.vector.tensor_scalar(
                        c[:, r], z[:, r], -1.0, 1.0, op0=OP.mult, op1=OP.add
                    )
                    # g = max(hidden + 0.5, sigmoid(hidden)), in place into hb
                    nc.vector.tensor_max(hb[:, r], hb[:, r], sh[:, r])
                    # v = z * g, in place into z
                    nc.vector.tensor_mul(z[:, r], z[:, r], hb[:, r])
                    init = 0.0 if first else carries[m][:]
                    nc.vector.tensor_tensor_scan(
                        h[:, r], c[:, r], z[:, r], init, op0=OP.mult, op1=OP.add
                    )
                    if not last_of_m:
                        nc.vector.tensor_copy(carries[m][:], h[:, w - 1 : w])
                    nc.sync.dma_start(
                        out=out_d[ts(m, P), ds(s0, w)], in_=h[:, r]
                    )

                # s-schedule: narrow chunks first (prologue needs less x
                # in flight), wide epochs in steady state
                if ew == 2 * QT:
                    s_chunks = [(0, QT), (QT, QT), (2 * QT, 2 * QT)]
                else:
                    s_chunks = [(q * ew, ew) for q in range(S // ew)]
                for ci, (cs0, cw) in enumerate(s_chunks):
                    for m in range(MB):
                        if ci == len(s_chunks) - 1 and m == MB - 1:
                            # taper the final row-block so the post-matmul
                            # tail is short
                            s0 = cs0
                            for w in ([QT] if cw == 2 * QT else []) + [NB, NB // 2, NB // 2]:
                                epoch_pair(m, s0, w, first=False,
                                           last_of_m=(s0 + w == S))
                                s0 += w
                        else:
                            epoch_pair(m, cs0, cw, first=(ci == 0),
                                       last_of_m=False)

            if bench_reps is None:
                body()
            else:
                with tc.For_i(
                    0, bench_reps, 1, hint_engines=(mybir.EngineType.PE,)
                ):
                    body()

    nc.compile()
    return nc


_NC = None


def _get_nc():
    global _NC
    if _NC is None:
        _NC = _build()
    return _NC


def _make_in_maps(x: np.ndarray, W: np.ndarray):
    xT16 = [np.ascontiguousarray(x[b].T).astype(np.float16) for b in range(B)]
    W16 = W.astype(np.float16)
    in_maps = []
    for core in range(N_CORES):
        b, dh = divmod(core, 2)
        d0 = dh * DSH
        in_maps.append(
            {
                "xT": xT16[b],
                "wh": np.ascontiguousarray(W16[:, d0 : d0 + DSH]),
                "wg": np.ascontiguousarray(W16[:, D + d0 : D + d0 + DSH]),
            }
        )
    return in_maps


def _gather(results):
    out = np.empty((B, S, D), dtype=np.float32)
    for core in range(N_CORES):
        b, dh = divmod(core, 2)
        d0 = dh * DSH
        out[b, :, d0 : d0 + DSH] = results[core]["out"].T
    next_prev_hidden = out[:, -1:, :].copy()
    return out, next_prev_hidden


def run(x: np.ndarray, W: np.ndarray, **spmd_kwargs):
    x = np.asarray(x)
    W = np.asarray(W)
    assert x.shape == (B, S, D) and W.shape == (D, 2 * D)
    nc = _get_nc()
    res = bass_utils.run_bass_kernel_spmd(
        nc, _make_in_maps(x, W), core_ids=list(range(N_CORES)), **spmd_kwargs
    )
    return res


def kernel(x: np.ndarray, W: np.ndarray):
    res = run(x, W)
    return _gather(res.results)
